# revision 20
# baseline (speedup 1.0000x reference)
"""Trainium2 Bass kernel for a 2-layer GATv2 + JumpingKnowledge GNN.

Strategy (8 NeuronCores, dst-node sharding, 3 launches, zero on-device
gathers):
  - Host: add self loops, bucket edges by (core, 128-node dst window).
    Windows are padded to a per-window-position block count bc_w
    (uniform across cores so one SPMD program serves all 8).
  - Launch A (node-sharded): xl1 = x@Wl1, xr1 = x@Wr1 + biases,
    jk0 = x@Wjk0 for owned nodes.  Pure per-node GEMMs.
  - Host: route xl1 rows into edge order (halo exchange): an edge-major
    bf16 copy (messages, for the alpha-weighted aggregation) and a
    feature-major fp8 copy (for the attention-logit pipeline), plus the
    per-window fp8 scatter one-hot g01t.  Pure permutation + fp8 cast.
  - Launch B: layer-1 edge phase + h1 + layer-2 node transforms.
  - Host: route xl2 the same way.
  - Launch C: layer-2 edge phase + JumpingKnowledge output projection.

Edge phase per superblock (up to 4 blocks of 128 edges):
  s_fm[g]  = DoubleRow fp8 matmul: [xr_win_g | I]^T @ [g01t | fm_g]
             (one PE pass computes xr[dst] + xl[src] per feature group)
  lr       = Prelu(s_fm, 0.2)                          (ACT/DVE)
  lg      += att_bd[g].T @ lr[g]                       (PE; layer 2: one
             fp8 DoubleRow matmul over both groups)
  expf     = Exp(lg); expe = transpose via tiny PE matmuls
  g01e[b]  = is_equal(iota128, dloc_col[b])            (DVE one-hot gen)
  pr[b]    = em[b] * expe[b]  (head-broadcast)         (DVE 4x mode)
  U       += g01e[b].T @ pr[b]; dn += g01e[b].T @ expe (PE; layer 2:
             fp8 DoubleRow over block pairs, denominator merged into U
             via ones columns baked into em)
Window epilogue: h = elu(U/dn + bias), then next-layer node GEMMs.

fp8 (e4m3) is used only where the ~2e-2 tolerance allows: the logit-path
operands (fm, xr, one-hots) in both layers, plus the whole layer-2 edge
phase (lr, att, expe, pr).  Messages (em) and layer-1 softmax stay bf16.

All feature axes use a head-interleaved order f=(c*H+h) so the DVE
broadcast multiply has innermost stride 1 (4x DVE perf mode).  Every
weight matrix is permuted accordingly on the host; the final output is
un-permuted (Wjk rows permuted to compensate).

The segment softmax skips the max subtraction: logits for this model are
in [-6, 6], exp() is safe, softmax is shift-invariant.
"""

import os
from contextlib import ExitStack

import ml_dtypes
import numpy as np

import concourse.bacc as bacc
import concourse.mybir as mybir
import concourse.tile as tile
from concourse.bass_utils import run_bass_kernel_spmd

dt = mybir.dt
AF = mybir.ActivationFunctionType
ALU = mybir.AluOpType
BF16 = ml_dtypes.bfloat16
F8 = ml_dtypes.float8_e4m3

# ---------------- problem constants (hardcoded per contract) ----------------
N = 20000
HID = 128
HEADS = 8
C1 = 64
C2 = 32
D1 = HEADS * C1  # 512
D2 = HEADS * C2  # 256
DE2 = D2 + 8     # em2 with ones columns (merged denominator)

NCORES = 8
NPC = N // NCORES          # 2500 nodes per core
WNODES = 128               # nodes per window
NW = -(-NPC // WNODES)     # 20 windows per core
NPAD = NW * WNODES         # 2560 padded node slots per core
BLK = 128                  # edges per block
E4 = 4 * BLK               # edge slots per full superblock

LAST_RESULTS = []          # BassKernelResults of the most recent kernel() call


def _bf(x):
    return np.ascontiguousarray(np.asarray(x, np.float32).astype(BF16))


def _f8(x):
    return np.ascontiguousarray(np.asarray(x, np.float32).astype(F8))


def _f32(x):
    return np.ascontiguousarray(np.asarray(x, np.float32))


def _perm(D, H):
    """Head-interleave permutation: interleaved col j holds original col
    (j%H)*C + j//H  (i.e. j = c*H + h)."""
    j = np.arange(D)
    return (j % H) * (D // H) + j // H


PERM1 = _perm(D1, HEADS)
PERM2 = _perm(D2, HEADS)


def _att_sg(att, D):
    """Sign block-diag lhsT for the logit DoubleRow matmuls: [128, nPair,
    2, 32] fp8 (+-1 entries, zero-padded beyond 8 heads)."""
    H, C = att.shape
    nG = D // 128
    bd = np.zeros((D, 32), np.float32)
    j = np.arange(D)
    bd[j, j % H] = np.sign(att[j % H, j // H])
    return bd.reshape(nG // 2, 2, 128, 32).transpose(2, 0, 1, 3)


def _aabs(att):
    """|att| per interleaved feature column f=(c*H+h)."""
    H, C = att.shape
    j = np.arange(H * C)
    return np.abs(att[j % H, j // H]).astype(np.float32)


def _nsb(bc):
    return -(-bc // 4)


def _plan_edges(edge_index):
    """Bucket self-loop-augmented edges by (core, window).  Block counts
    bc_w are maxed over cores per window position (same SPMD program).
    Returns (bcs, srcs, dlocs) where
      bcs        = [NW] blocks per window
      srcs[c]    = [NW][bc_w*128] int64 src per slot (-1 for pads)
      dlocs[c]   = [NW][bc_w*128] int16 dst-in-window (-1 for pads)"""
    src = np.concatenate([edge_index[0].astype(np.int64),
                          np.arange(N, dtype=np.int64)])
    dst = np.concatenate([edge_index[1].astype(np.int64),
                          np.arange(N, dtype=np.int64)])
    core = dst // NPC
    dloc = dst - core * NPC
    win = dloc // WNODES
    din = dloc % WNODES

    order = np.lexsort((win, core))
    src, core, win, din = src[order], core[order], win[order], din[order]

    lists = {}
    cnts = np.zeros((NCORES, NW), np.int64)
    for c in range(NCORES):
        mc = core == c
        sc, wc, dc = src[mc], win[mc], din[mc]
        for w in range(NW):
            mw = wc == w
            lists[(c, w)] = (sc[mw], dc[mw])
            cnts[c, w] = mw.sum()
    bcs = [-(-int(cnts[:, w].max()) // BLK) for w in range(NW)]

    srcs, dlocs = [], []
    for c in range(NCORES):
        sl, dl = [], []
        for w in range(NW):
            ns = bcs[w] * BLK
            sp = np.full(ns, -1, np.int64)
            dp = np.full(ns, -1, np.int16)
            s_, d_ = lists[(c, w)]
            sp[:len(s_)] = s_
            dp[:len(d_)] = d_
            sl.append(sp)
            dl.append(dp)
        srcs.append(sl)
        dlocs.append(dl)
    return bcs, srcs, dlocs


def _route_edges(table_bf, srcs, bcs, DE, with_ones):
    """Gather table rows into edge order, per core.

    table_bf: [N, D] bf16 (feature cols already head-interleaved)
    returns (em, fm) lists per core:
      em[c]: [128, sum_w bc_w*DE] bf16, per block [128 e, DE] edge-major
             (with_ones: last 8 cols per block are 1.0; pads all-zero)
      fm[c]: [128, sum_w bc_w*nG*128... ] fp8: per sb [g01t | fm_g ...]
             is built separately in _build_sfx."""
    D = table_bf.shape[1]
    out = []
    for sl in srcs:
        pieces = []
        for w, sp in enumerate(sl):
            gat = np.zeros((len(sp), DE), BF16)
            real = sp >= 0
            gat[real, :D] = table_bf[sp[real]]
            if with_ones:
                gat[real, D:] = np.float32(1.0)
            # em[p, blk*DE + f] = gat[blk*128+p, f]
            em = gat.reshape(bcs[w], BLK, DE).transpose(1, 0, 2)
            pieces.append(np.ascontiguousarray(em).reshape(BLK, bcs[w] * DE))
        out.append(np.concatenate(pieces, axis=1))
    return out


def _build_sfx(table_f8, srcs, dlocs, bcs, nG):
    """Per-sb fp8 [g01t | fm_0 .. fm_{nG-1}] stream, per core.

    Row layout per partition p: for each (w, sb):
      [g01t[p, e], fm_0[p, e], ..., fm_{nG-1}[p, e]]  each nblk*128 wide
    g01t[n, e] = (dloc[e] == n);  fm_g[j, e] = table[src[e], g*128+j]."""
    out = []
    for sl, dl in zip(srcs, dlocs):
        pieces = []
        for w, (sp, dp) in enumerate(zip(sl, dl)):
            bc = bcs[w]
            for sb in range(_nsb(bc)):
                nblk = min(4, bc - 4 * sb)
                ne = nblk * BLK
                lo = 4 * sb * BLK
                spe, dpe = sp[lo:lo + ne], dp[lo:lo + ne]
                sfx = np.zeros((BLK, (1 + nG) * ne), F8)
                e = np.arange(ne)
                real = dpe >= 0
                sfx[dpe[real], e[real]] = np.float32(1.0)
                gat = np.zeros((ne, nG * 128), F8)
                realm = spe >= 0
                gat[realm] = table_f8[spe[realm]]
                # fm_g[j, e] = gat[e, g*128+j]
                fmt = gat.reshape(ne, nG, 128).transpose(2, 1, 0)
                sfx[:, ne:] = np.ascontiguousarray(fmt).reshape(128, nG * ne)
                pieces.append(sfx)
        out.append(np.concatenate(pieces, axis=1))
    return out


def _build_g01e(dlocs, bcs, np_dt):
    """Per-block [128 e, 128 n] one-hot (edge-major), concatenated per
    core: [128, total_blocks*128]."""
    out = []
    for dl in dlocs:
        pieces = []
        for w, dp in enumerate(dl):
            g = np.zeros((bcs[w], BLK, BLK), np_dt)
            blk = np.arange(len(dp)) // BLK
            pin = np.arange(len(dp)) % BLK
            real = dp >= 0
            g[blk[real], pin[real], dp[real]] = np.float32(1.0)
            # [p, blk*128 + n]
            pieces.append(np.ascontiguousarray(g.transpose(1, 0, 2))
                          .reshape(BLK, bcs[w] * BLK))
        out.append(np.concatenate(pieces, axis=1))
    return out


def _build_dcol(dlocs, bcs):
    """[NW, 128, 20] bf16 dst-in-window per (block, slot): dcol[w, p, b] =
    dloc of edge slot b*128+p (or -1 pad)."""
    out = []
    for dl in dlocs:
        dcol = np.full((NW, BLK, 20), -1.0, BF16)
        for w, dp in enumerate(dl):
            bc = bcs[w]
            dcol[w, :, :bc] = dp.reshape(bc, BLK).T.astype(BF16)
        out.append(dcol)
    return out


def _build_xrl(xr_f8, nG, with_i):
    """[NW, 128, nG, 2, 128] fp8 DoubleRow lhsT: k-tile0 = xr_win group g,
    k-tile1 = identity."""
    ident = np.eye(128, dtype=F8)
    out = np.zeros((NW, 128, nG, 2, 128), F8)
    for w in range(NW):
        xw = xr_f8[w * 128:(w + 1) * 128]          # [128 n, D]
        out[w, :, :, 0, :] = xw.reshape(128, nG, 128)
        out[w, :, :, 1, :] = ident[:, None, :]
    return np.ascontiguousarray(out)


# ------------------------------ launch A -----------------------------------

def _build_launch_a():
    nc = bacc.Bacc(None, target_bir_lowering=False)
    x_ownT = nc.dram_tensor("x_ownT", [128, NPAD], dt.bfloat16,
                            kind="ExternalInput")
    Wl1p = nc.dram_tensor("Wl1p", [128, D1], dt.bfloat16, kind="ExternalInput")
    Wr1p = nc.dram_tensor("Wr1p", [128, D1], dt.bfloat16, kind="ExternalInput")
    bxr1p = nc.dram_tensor("bxr1p", [128, D1], dt.float32, kind="ExternalInput")
    Wjk0 = nc.dram_tensor("Wjk0", [128, 128], dt.bfloat16, kind="ExternalInput")

    ao = nc.dram_tensor("ao", [NPAD, 2 * D1 + 128], dt.bfloat16,
                        kind="ExternalOutput")

    with tile.TileContext(nc) as tc, ExitStack() as ctx:
        const = ctx.enter_context(tc.tile_pool(name="const", bufs=1))
        sbuf = ctx.enter_context(tc.tile_pool(name="sbuf", bufs=5))
        pp = ctx.enter_context(tc.tile_pool(name="pp", bufs=6, space="PSUM"))
        pps = ctx.enter_context(tc.tile_pool(name="pps", bufs=2, space="PSUM"))

        def cl(name, hdl, shape, dtype):
            t = const.tile(shape, dtype, tag=name)
            nc.sync.dma_start(t[:], hdl[:])
            return t

        wl = cl("wl", Wl1p, [128, D1], dt.bfloat16)
        wr = cl("wr", Wr1p, [128, D1], dt.bfloat16)
        bx = cl("bx", bxr1p, [128, D1], dt.float32)
        wj = cl("wj", Wjk0, [128, 128], dt.bfloat16)
        xo = const.tile([128, NPAD], dt.bfloat16, tag="xo")
        nc.sync.dma_start(xo[:], x_ownT[:])

        outq = [nc.gpsimd, nc.gpsimd, nc.gpsimd]
        for w in range(NW):
            lhs = xo[:, w * 128:(w + 1) * 128]
            t1 = sbuf.tile([128, 2 * D1 + 128], dt.bfloat16, tag="t1")
            p1 = pp.tile([128, D1], dt.float32, tag="p1")
            nc.tensor.matmul(p1[:], lhsT=lhs, rhs=wl[:], start=True, stop=True)
            nc.vector.tensor_copy(t1[:, :D1], p1[:])
            p2 = pp.tile([128, D1], dt.float32, tag="p1")
            nc.tensor.matmul(p2[:], lhsT=lhs, rhs=wr[:], start=True, stop=True)
            nc.vector.tensor_tensor(out=t1[:, D1:2 * D1], in0=p2[:], in1=bx[:],
                                    op=ALU.add)
            p3 = pps.tile([128, 128], dt.float32, tag="p3")
            nc.tensor.matmul(p3[:], lhsT=lhs, rhs=wj[:], start=True, stop=True)
            nc.gpsimd.dma_start(ao[w * 128:(w + 1) * 128, :2 * D1],
                                t1[:, :2 * D1])
            nc.scalar.activation(t1[:, 2 * D1:], p3[:], AF.Copy)
            outq[w % 3].dma_start(ao[w * 128:(w + 1) * 128, 2 * D1:],
                                  t1[:, 2 * D1:])

    nc.compile()
    return nc


# ------------------------- edge-phase launches ------------------------------

def _sfx_offsets(bcs, nG):
    """Per-(w, sb) column offset of the sfx stream; total width."""
    offs = {}
    o = 0
    for w, bc in enumerate(bcs):
        for sb in range(_nsb(bc)):
            nblk = min(4, bc - 4 * sb)
            offs[(w, sb)] = o
            o += (1 + nG) * nblk * BLK
    return offs, o


def _em_offsets(bcs, DE):
    offs = {}
    o = 0
    for w, bc in enumerate(bcs):
        for sb in range(_nsb(bc)):
            nblk = min(4, bc - 4 * sb)
            offs[(w, sb)] = o
            o += nblk * DE
    return offs, o


def _build_launch_b(bcs):
    """Layer-1 edge phase: s via fp8 DoubleRow, lg/U/dn bf16, plus per-
    window h1 epilogue and layer-2 node transforms."""
    nG = 4
    sfx_off, sfx_w = _sfx_offsets(bcs, nG)
    em_off, em_w = _em_offsets(bcs, D1)
    nc = bacc.Bacc(None, target_bir_lowering=False)

    em_d = nc.dram_tensor("em", [128, em_w], dt.bfloat16, kind="ExternalInput")
    sfx_d = nc.dram_tensor("sfx", [128, sfx_w], dt.float8e4,
                           kind="ExternalInput")
    xrl_d = nc.dram_tensor("xrl", [NW, 128, nG * 2 * 128], dt.float8e4,
                           kind="ExternalInput")
    g01e_d = nc.dram_tensor("g01e", [128, sum(bcs) * BLK], dt.bfloat16,
                            kind="ExternalInput")
    jk0_d = nc.dram_tensor("jk0", [NW, 128, 128], dt.bfloat16,
                           kind="ExternalInput")
    att1sg = nc.dram_tensor("att1sg", [128, 128], dt.float8e4,
                            kind="ExternalInput")
    biash1 = nc.dram_tensor("biash1", [128, D1], dt.float32,
                            kind="ExternalInput")
    identI = nc.dram_tensor("identI", [128, 128], dt.bfloat16,
                            kind="ExternalInput")
    Wlr2p = nc.dram_tensor("Wlr2p", [128, 4 * 2 * D2], dt.bfloat16,
                           kind="ExternalInput")
    bxr2p = nc.dram_tensor("bxr2p", [128, D2], dt.float32,
                           kind="ExternalInput")
    Wjk1p = nc.dram_tensor("Wjk1p", [128, 4 * 128], dt.bfloat16,
                           kind="ExternalInput")

    xl2_o = nc.dram_tensor("xl2_o", [NPAD, D2], dt.bfloat16,
                           kind="ExternalOutput")
    xr2_o = nc.dram_tensor("xr2_o", [NPAD, D2], dt.bfloat16,
                           kind="ExternalOutput")
    jk01_o = nc.dram_tensor("jk01_o", [NPAD, 128], dt.float32,
                            kind="ExternalOutput")

    with tile.TileContext(nc) as tc, ExitStack() as ctx:
        const = ctx.enter_context(tc.tile_pool(name="const", bufs=1))
        sbuf = ctx.enter_context(tc.tile_pool(name="sbuf", bufs=3))
        empool = ctx.enter_context(tc.tile_pool(name="em", bufs=3))
        sfxpool = ctx.enter_context(tc.tile_pool(name="sfx", bufs=3))
        lrpool = ctx.enter_context(tc.tile_pool(name="lr", bufs=2))
        prpool = ctx.enter_context(tc.tile_pool(name="pr", bufs=2))
        wpool = ctx.enter_context(tc.tile_pool(name="wp", bufs=2))
        gepool = ctx.enter_context(tc.tile_pool(name="ge", bufs=4))
        ppS = ctx.enter_context(tc.tile_pool(name="ppS", bufs=2, space="PSUM"))
        ppLG = ctx.enter_context(tc.tile_pool(name="ppLG", bufs=1, space="PSUM"))
        ppE = ctx.enter_context(tc.tile_pool(name="ppE", bufs=1, space="PSUM"))
        ppU = ctx.enter_context(tc.tile_pool(name="ppU", bufs=2, space="PSUM"))
        ppDN = ctx.enter_context(tc.tile_pool(name="ppDN", bufs=2, space="PSUM"))

        def cl(name, hdl, shape, dtype, eng=None):
            t = const.tile(shape, dtype, tag=name)
            dst = t[:]
            if len(shape) > 2:
                dst = dst.rearrange("p ... -> p (...)")
            (eng or nc.sync).dma_start(dst, hdl[:])
            return t

        att1_t = cl("att1", att1sg, [128, 2, 2, 32], dt.float8e4)
        ident = cl("ident", identI, [128, 128], dt.bfloat16)
        bh1_t = const.tile([128, D1], dt.float32, tag="bh1")
        wlr2_t = const.tile([128, 4, 2 * D2], dt.bfloat16, tag="wlr2")
        bxr2_t = const.tile([128, D2], dt.float32, tag="bxr2")
        wjk1_t = const.tile([128, 4 * 128], dt.bfloat16, tag="wjk1")

        def late_consts():
            nc.gpsimd.dma_start(bh1_t[:], biash1[:])
            nc.gpsimd.dma_start(
                wlr2_t[:].rearrange("p a e -> p (a e)"), Wlr2p[:])
            nc.gpsimd.dma_start(bxr2_t[:], bxr2p[:])
            nc.gpsimd.dma_start(wjk1_t[:], Wjk1p[:])

        state = {}
        g01e_off = {}
        _o = 0
        for _w, _bc in enumerate(bcs):
            for _sb in range(_nsb(_bc)):
                g01e_off[(_w, _sb)] = _o
                _o += min(4, _bc - 4 * _sb) * BLK

        def prefetch(w, sb):
            nblk = min(4, bcs[w] - 4 * sb)
            if sb == 0:
                xrl_t = wpool.tile([128, nG, 2, 128], dt.float8e4, tag="xrl")
                nc.gpsimd.dma_start(
                    xrl_t[:].rearrange("p a b c -> p (a b c)"), xrl_d[w])
                state[("win", w)] = xrl_t
            sfx_t = sfxpool.tile([128, 1 + nG, E4], dt.float8e4, tag="sfx")
            so = sfx_off[(w, sb)]
            nc.sync.dma_start(
                sfx_t[:, :, :nblk * BLK],
                sfx_d[:, so:so + (1 + nG) * nblk * BLK]
                .rearrange("p (a e) -> p a e", a=1 + nG))
            em_t = empool.tile([128, 4, D1], dt.bfloat16, tag="em")
            eo = em_off[(w, sb)]
            nc.sync.dma_start(
                em_t[:, :nblk, :],
                em_d[:, eo:eo + nblk * D1]
                .rearrange("p (a e) -> p a e", a=nblk))
            g01e_t = gepool.tile([128, 4, BLK], dt.bfloat16, tag="g01e")
            go = g01e_off[(w, sb)]
            nc.sync.dma_start(
                g01e_t[:, :nblk, :],
                g01e_d[:, go:go + nblk * BLK]
                .rearrange("p (a e) -> p a e", a=nblk))
            return sfx_t, em_t, g01e_t

        def phase1(w, sb, tiles):
            sfx_t, em_t, g01e_t = tiles
            nblk = min(4, bcs[w] - 4 * sb)
            ne = nblk * BLK
            xrl_t = state[("win", w)]
            lg = ppLG.tile([32, E4], dt.float32, tag="lg")
            lr = lrpool.tile([128, nG, E4], dt.float8e4, tag="lr")
            for g in range(nG):
                s = ppS.tile([128, E4], dt.float32, tag="s")
                nc.tensor.matmul(
                    s[:, :ne],
                    lhsT=xrl_t[:, g, :, :],
                    rhs=sfx_t[:, 0:(2 + g):(1 + g), :ne],
                    start=True, stop=True,
                    perf_mode=mybir.MatmulPerfMode.DoubleRow)
                nc.scalar.activation(lr[:, g, :ne], s[:, :ne],
                                     AF.Prelu, alpha=0.2)
                if g == 1:
                    nc.tensor.matmul(
                        lg[:, :ne], lhsT=att1_t[:, 0, :, :],
                        rhs=lr[:, 0:2, :ne], start=True, stop=False,
                        perf_mode=mybir.MatmulPerfMode.DoubleRow)
            nc.tensor.matmul(
                lg[:, :ne], lhsT=att1_t[:, 1, :, :],
                rhs=lr[:, 2:4, :ne], start=False, stop=True,
                perf_mode=mybir.MatmulPerfMode.DoubleRow)
            return lg

        def emit_exp(w, sb, lg):
            nblk = min(4, bcs[w] - 4 * sb)
            expf = sbuf.tile([8, E4], dt.bfloat16, tag="expf")
            nc.scalar.activation(expf[:, :nblk * BLK], lg[:8, :nblk * BLK],
                                 AF.Exp)
            return expf

        def phase2a(w, sb, tiles, expf):
            _, em_t, g01e_t = tiles
            nblk = min(4, bcs[w] - 4 * sb)
            ept = ppE.tile([128, 32], dt.float32, tag="ept")
            for b in range(nblk):
                nc.tensor.matmul(ept[:, b * 8:(b + 1) * 8],
                                 lhsT=expf[:, b * BLK:(b + 1) * BLK],
                                 rhs=ident[:8, :8],
                                 start=(b == 0), stop=(b == nblk - 1))
            expe = sbuf.tile([128, 32], dt.bfloat16, tag="expe")
            nc.scalar.activation(expe[:, :nblk * 8], ept[:, :nblk * 8],
                                 AF.Copy)
            pr = prpool.tile([128, 4, D1], dt.bfloat16, tag="pr")
            for b in range(nblk):
                nc.vector.tensor_tensor(
                    out=pr[:, b, :].rearrange("p (c h) -> p c h", h=8),
                    in0=em_t[:, b, :].rearrange("p (c h) -> p c h", h=8),
                    in1=expe[:, b * 8:(b + 1) * 8].unsqueeze(1)
                        .broadcast_to([128, C1, 8]),
                    op=ALU.mult)
            return pr, expe, g01e_t

        def phase2b(w, sb, pr, expe, g01e):
            nblk = min(4, bcs[w] - 4 * sb)
            U, dn = state[("U", w)]
            nsb = _nsb(bcs[w])
            for b in range(nblk):
                first = (sb == 0 and b == 0)
                last = (sb == nsb - 1 and b == nblk - 1)
                nc.tensor.matmul(U[:], lhsT=g01e[:, b, :], rhs=pr[:, b, :],
                                 start=first, stop=last)
                nc.tensor.matmul(dn[:], lhsT=g01e[:, b, :],
                                 rhs=expe[:, b * 8:(b + 1) * 8],
                                 start=first, stop=last)

        def begin_window(w):
            U = ppU.tile([128, D1], dt.float32, tag="U")
            dn = ppDN.tile([128, 8], dt.float32, tag="dn")
            state[("U", w)] = (U, dn)

        def epi_v(w):
            U, dn = state[("U", w)]
            dns = sbuf.tile([128, 8], dt.float32, tag="dns")
            nc.vector.tensor_scalar_max(dns[:], dn[:], 1e-30)
            rd = sbuf.tile([128, 8], dt.float32, tag="rd")
            nc.vector.reciprocal(rd[:], dns[:])
            v = sbuf.tile([128, D1], dt.float32, tag="v")
            nc.vector.tensor_tensor(
                out=v[:].rearrange("p (c h) -> p c h", h=8),
                in0=U[:].rearrange("p (c h) -> p c h", h=8),
                in1=rd[:].unsqueeze(1).broadcast_to([128, C1, 8]),
                op=ALU.mult)
            vb = sbuf.tile([128, D1], dt.float32, tag="vb")
            nc.vector.tensor_tensor(out=vb[:], in0=v[:], in1=bh1_t[:],
                                    op=ALU.add)
            m = sbuf.tile([128, D1], dt.float32, tag="m")
            nc.vector.tensor_scalar_min(m[:], vb[:], 0.0)
            em_ = sbuf.tile([128, D1], dt.float32, tag="em_")
            nc.scalar.activation(em_[:], m[:], AF.Exp)
            h = sbuf.tile([128, D1], dt.bfloat16, tag="h")
            nc.vector.scalar_tensor_tensor(out=h[:], in0=em_[:], scalar=-1.0,
                                           op0=ALU.add, in1=vb[:], op1=ALU.max)
            state[("h", w)] = h

        def on_h(w, h):
            # [xl2 | xr2] = h@[Wl2p Wr2p] ; jk01 = jk0 + h@Wjk1p
            p_xlr = ppS.tile([128, 2 * D2], dt.float32, tag="s")
            p_jk = ppE.tile([128, 128], dt.float32, tag="ept")
            for g in range(4):
                tp = ppLG.tile([128, 128], dt.float32, tag="lg")
                nc.tensor.matmul(tp[:], lhsT=h[:, g * 128:(g + 1) * 128],
                                 rhs=ident[:], start=True, stop=True)
                hTs = sbuf.tile([128, 128], dt.bfloat16, tag="hT")
                nc.vector.tensor_copy(hTs[:], tp[:])
                nc.tensor.matmul(p_xlr[:], lhsT=hTs[:],
                                 rhs=wlr2_t[:, g, :],
                                 start=(g == 0), stop=(g == 3))
                nc.tensor.matmul(p_jk[:], lhsT=hTs[:],
                                 rhs=wjk1_t[:, g * 128:(g + 1) * 128],
                                 start=(g == 0), stop=(g == 3))
            o_xl2 = sbuf.tile([128, D2], dt.bfloat16, tag="oxl2")
            nc.any.tensor_copy(o_xl2[:], p_xlr[:, :D2])
            nc.gpsimd.dma_start(xl2_o[w * 128:(w + 1) * 128, :], o_xl2[:])
            o_xr2 = sbuf.tile([128, D2], dt.bfloat16, tag="oxr2")
            nc.vector.tensor_tensor(out=o_xr2[:], in0=p_xlr[:, D2:],
                                    in1=bxr2_t[:], op=ALU.add)
            nc.gpsimd.dma_start(xr2_o[w * 128:(w + 1) * 128, :], o_xr2[:])
            jk0_t = sbuf.tile([128, 128], dt.bfloat16, tag="jk0")
            nc.gpsimd.dma_start(jk0_t[:], jk0_d[w])
            o_jk = sbuf.tile([128, 128], dt.float32, tag="ojk")
            nc.vector.tensor_tensor(out=o_jk[:], in0=p_jk[:], in1=jk0_t[:],
                                    op=ALU.add)
            nc.gpsimd.dma_start(jk01_o[w * 128:(w + 1) * 128, :], o_jk[:])

        _run_pipeline(bcs, prefetch, begin_window, phase1, emit_exp,
                      phase2a, phase2b, epi_v, on_h, state, late_consts)

    nc.compile()
    return nc


def _build_launch_c(bcs):
    """Layer-2 edge phase: everything fp8 DoubleRow (s, lg, U with merged
    denominator), plus the final JK projection."""
    nG = 2
    sfx_off, sfx_w = _sfx_offsets(bcs, nG)
    em_off, em_w = _em_offsets(bcs, DE2)
    nc = bacc.Bacc(None, target_bir_lowering=False)

    em_d = nc.dram_tensor("em", [128, em_w], dt.bfloat16, kind="ExternalInput")
    sfx_d = nc.dram_tensor("sfx", [128, sfx_w], dt.float8e4,
                           kind="ExternalInput")
    xrl_d = nc.dram_tensor("xrl", [NW, 128, nG * 2 * 128], dt.float8e4,
                           kind="ExternalInput")
    g01e_d = nc.dram_tensor("g01e", [128, sum(bcs) * BLK], dt.float8e4,
                            kind="ExternalInput")
    jk01_d = nc.dram_tensor("jk01", [NW, 128, 128], dt.float32,
                            kind="ExternalInput")
    att2sg = nc.dram_tensor("att2sg", [128, 64], dt.float8e4,
                            kind="ExternalInput")
    biash2 = nc.dram_tensor("biash2", [128, D2], dt.float32,
                            kind="ExternalInput")
    identI = nc.dram_tensor("identI", [128, 128], dt.bfloat16,
                            kind="ExternalInput")
    ident8f = nc.dram_tensor("ident8f", [8, 8], dt.float8e4,
                             kind="ExternalInput")
    Wjk2p = nc.dram_tensor("Wjk2p", [128, 2 * 128], dt.bfloat16,
                           kind="ExternalInput")

    out_o = nc.dram_tensor("out_o", [NPAD, 128], dt.float32,
                           kind="ExternalOutput")

    with tile.TileContext(nc) as tc, ExitStack() as ctx:
        const = ctx.enter_context(tc.tile_pool(name="const", bufs=1))
        sbuf = ctx.enter_context(tc.tile_pool(name="sbuf", bufs=3))
        empool = ctx.enter_context(tc.tile_pool(name="em", bufs=3))
        sfxpool = ctx.enter_context(tc.tile_pool(name="sfx", bufs=3))
        lrpool = ctx.enter_context(tc.tile_pool(name="lr", bufs=2))
        prpool = ctx.enter_context(tc.tile_pool(name="pr", bufs=2))
        wpool = ctx.enter_context(tc.tile_pool(name="wp", bufs=2))
        gepool = ctx.enter_context(tc.tile_pool(name="ge", bufs=4))
        ppS = ctx.enter_context(tc.tile_pool(name="ppS", bufs=2, space="PSUM"))
        ppLG = ctx.enter_context(tc.tile_pool(name="ppLG", bufs=1, space="PSUM"))
        ppE = ctx.enter_context(tc.tile_pool(name="ppE", bufs=1, space="PSUM"))
        ppU = ctx.enter_context(tc.tile_pool(name="ppU", bufs=2, space="PSUM"))

        def cl(name, hdl, shape, dtype):
            t = const.tile(shape, dtype, tag=name)
            nc.sync.dma_start(t[:], hdl[:])
            return t

        att2_t = cl("att2", att2sg, [128, 64], dt.float8e4)
        ident = cl("ident", identI, [128, 128], dt.bfloat16)
        id8f = cl("id8f", ident8f, [8, 8], dt.float8e4)
        bh2_t = const.tile([128, D2], dt.float32, tag="bh2")
        wjk2_t = const.tile([128, 2 * 128], dt.bfloat16, tag="wjk2")

        def late_consts():
            nc.gpsimd.dma_start(bh2_t[:], biash2[:])
            nc.gpsimd.dma_start(wjk2_t[:], Wjk2p[:])

        state = {}
        g01e_off = {}
        _o = 0
        for _w, _bc in enumerate(bcs):
            for _sb in range(_nsb(_bc)):
                g01e_off[(_w, _sb)] = _o
                _o += min(4, _bc - 4 * _sb) * BLK

        def prefetch(w, sb):
            nblk = min(4, bcs[w] - 4 * sb)
            if sb == 0:
                xrl_t = wpool.tile([128, nG, 2, 128], dt.float8e4, tag="xrl")
                nc.gpsimd.dma_start(
                    xrl_t[:].rearrange("p a b c -> p (a b c)"), xrl_d[w])
                state[("win", w)] = xrl_t
            sfx_t = sfxpool.tile([128, 1 + nG, E4], dt.float8e4, tag="sfx")
            so = sfx_off[(w, sb)]
            nc.sync.dma_start(
                sfx_t[:, :, :nblk * BLK],
                sfx_d[:, so:so + (1 + nG) * nblk * BLK]
                .rearrange("p (a e) -> p a e", a=1 + nG))
            em_t = empool.tile([128, 4, DE2], dt.bfloat16, tag="em")
            eo = em_off[(w, sb)]
            nc.sync.dma_start(
                em_t[:, :nblk, :],
                em_d[:, eo:eo + nblk * DE2]
                .rearrange("p (a e) -> p a e", a=nblk))
            g01e_t = gepool.tile([128, 4, BLK], dt.float8e4, tag="g01e")
            go = g01e_off[(w, sb)]
            nc.sync.dma_start(
                g01e_t[:, :nblk, :],
                g01e_d[:, go:go + nblk * BLK]
                .rearrange("p (a e) -> p a e", a=nblk))
            return sfx_t, em_t, g01e_t

        def phase1(w, sb, tiles):
            sfx_t, em_t, g01e_t = tiles
            nblk = min(4, bcs[w] - 4 * sb)
            ne = nblk * BLK
            xrl_t = state[("win", w)]
            lr = lrpool.tile([128, nG, E4], dt.float8e4, tag="lr")
            for g in range(nG):
                s = ppS.tile([128, E4], dt.float32, tag="s")
                nc.tensor.matmul(
                    s[:, :ne],
                    lhsT=xrl_t[:, g, :, :],
                    rhs=sfx_t[:, 0:(2 + g):(1 + g), :ne],
                    start=True, stop=True,
                    perf_mode=mybir.MatmulPerfMode.DoubleRow)
                nc.scalar.activation(lr[:, g, :ne], s[:, :ne],
                                     AF.Prelu, alpha=0.2)
            lg = ppLG.tile([32, E4], dt.float32, tag="lg")
            nc.tensor.matmul(lg[:, :ne], lhsT=att2_t[:].rearrange(
                                 "p (a b) -> p a b", a=2),
                             rhs=lr[:, :, :ne], start=True, stop=True,
                             perf_mode=mybir.MatmulPerfMode.DoubleRow)
            return lg

        def emit_exp(w, sb, lg):
            nblk = min(4, bcs[w] - 4 * sb)
            expf = sbuf.tile([8, E4], dt.float8e4, tag="expf")
            nc.scalar.activation(expf[:, :nblk * BLK], lg[:8, :nblk * BLK],
                                 AF.Exp)
            return expf

        def phase2a(w, sb, tiles, expf):
            _, em_t, g01e_t = tiles
            nblk = min(4, bcs[w] - 4 * sb)
            ept = ppE.tile([128, 128], dt.float32, tag="ept")
            for b in range(nblk):
                nc.tensor.matmul(ept[:, b * 8:(b + 1) * 8],
                                 lhsT=expf[:, b * BLK:(b + 1) * BLK],
                                 rhs=id8f[:],
                                 start=(b == 0), stop=(b == nblk - 1))
            expe = sbuf.tile([128, 32], dt.float8e4, tag="expe")
            nc.scalar.activation(expe[:, :nblk * 8], ept[:, :nblk * 8],
                                 AF.Copy)
            pr = prpool.tile([128, 4, DE2], dt.float8e4, tag="pr")
            for b in range(nblk):
                nc.vector.tensor_tensor(
                    out=pr[:, b, :].rearrange("p (c h) -> p c h", h=8),
                    in0=em_t[:, b, :].rearrange("p (c h) -> p c h", h=8),
                    in1=expe[:, b * 8:(b + 1) * 8].unsqueeze(1)
                        .broadcast_to([128, C2 + 1, 8]),
                    op=ALU.mult)
            return pr, expe, g01e_t

        def phase2b(w, sb, pr, expe, g01e):
            nblk = min(4, bcs[w] - 4 * sb)
            U, _ = state[("U", w)]
            nsb = _nsb(bcs[w])
            b = 0
            while b < nblk:
                first = (sb == 0 and b == 0)
                if b + 1 < nblk:
                    last = (sb == nsb - 1 and b + 2 >= nblk)
                    nc.tensor.matmul(
                        U[:], lhsT=g01e[:, b:b + 2, :], rhs=pr[:, b:b + 2, :],
                        start=first, stop=last,
                        perf_mode=mybir.MatmulPerfMode.DoubleRow)
                    b += 2
                else:
                    last = (sb == nsb - 1)
                    nc.tensor.matmul(U[:], lhsT=g01e[:, b, :],
                                     rhs=pr[:, b, :], start=first, stop=last)
                    b += 1

        def begin_window(w):
            U = ppU.tile([128, DE2], dt.float32, tag="U")
            state[("U", w)] = (U, None)

        def epi_v(w):
            U, _ = state[("U", w)]
            dns = sbuf.tile([128, 8], dt.float32, tag="dns")
            nc.vector.tensor_scalar_max(dns[:], U[:, D2:], 1e-30)
            rd = sbuf.tile([128, 8], dt.float32, tag="rd")
            nc.vector.reciprocal(rd[:], dns[:])
            v = sbuf.tile([128, D2], dt.float32, tag="v")
            nc.vector.tensor_tensor(
                out=v[:].rearrange("p (c h) -> p c h", h=8),
                in0=U[:, :D2].rearrange("p (c h) -> p c h", h=8),
                in1=rd[:].unsqueeze(1).broadcast_to([128, C2, 8]),
                op=ALU.mult)
            vb = sbuf.tile([128, D2], dt.float32, tag="vb")
            nc.vector.tensor_tensor(out=vb[:], in0=v[:], in1=bh2_t[:],
                                    op=ALU.add)
            m = sbuf.tile([128, D2], dt.float32, tag="m")
            nc.vector.tensor_scalar_min(m[:], vb[:], 0.0)
            em_ = sbuf.tile([128, D2], dt.float32, tag="em_")
            nc.scalar.activation(em_[:], m[:], AF.Exp)
            h = sbuf.tile([128, D2], dt.bfloat16, tag="h")
            nc.vector.scalar_tensor_tensor(out=h[:], in0=em_[:], scalar=-1.0,
                                           op0=ALU.add, in1=vb[:], op1=ALU.max)
            state[("h", w)] = h

        def on_h(w, h):
            p_out = ppS.tile([128, 128], dt.float32, tag="s")
            for g in range(2):
                tp = ppLG.tile([128, 128], dt.float32, tag="lg")
                nc.tensor.matmul(tp[:], lhsT=h[:, g * 128:(g + 1) * 128],
                                 rhs=ident[:], start=True, stop=True)
                hTs = sbuf.tile([128, 128], dt.bfloat16, tag="hT")
                nc.vector.tensor_copy(hTs[:], tp[:])
                nc.tensor.matmul(p_out[:], lhsT=hTs[:],
                                 rhs=wjk2_t[:, g * 128:(g + 1) * 128],
                                 start=(g == 0), stop=(g == 1))
            jk_t = sbuf.tile([128, 128], dt.float32, tag="jkt")
            nc.gpsimd.dma_start(jk_t[:], jk01_d[w])
            o_t = sbuf.tile([128, 128], dt.float32, tag="ot")
            nc.vector.tensor_tensor(out=o_t[:], in0=p_out[:], in1=jk_t[:],
                                    op=ALU.add)
            nc.gpsimd.dma_start(out_o[w * 128:(w + 1) * 128, :], o_t[:])

        _run_pipeline(bcs, prefetch, begin_window, phase1, emit_exp,
                      phase2a, phase2b, epi_v, on_h, state, late_consts)

    nc.compile()
    return nc


def _run_pipeline(bcs, prefetch, begin_window, phase1, emit_exp, phase2a,
                  phase2b, epi_v, on_h, state, late_consts):
    """Software pipeline: phase2a lags phase1 by 1 superblock, phase2b by
    3, window epilogue 2 iterations after the last phase2b."""
    items = [(w, sb) for w in range(NW) for sb in range(_nsb(bcs[w]))]
    tiles_q = []
    fifo_a, fifo_b = [], []
    epi_cd = []

    tiles_q.append(prefetch(*items[0]))

    def tick():
        if len(fifo_b) > 2:
            w, sb, pr, expe, g01e = fifo_b.pop(0)
            phase2b(w, sb, pr, expe, g01e)
            if sb == _nsb(bcs[w]) - 1:
                epi_v(w)
                epi_cd.append([w, 2])
        for e in epi_cd:
            e[1] -= 1
        while epi_cd and epi_cd[0][1] <= 0:
            w = epi_cd.pop(0)[0]
            on_h(w, state.pop(("h", w)))
            del state[("win", w)], state[("U", w)]

    for i, (w, sb) in enumerate(items):
        if sb == 0:
            begin_window(w)
        if i + 1 < len(items):
            tiles_q.append(prefetch(*items[i + 1]))
        tiles = tiles_q.pop(0)
        lg = phase1(w, sb, tiles)
        if i == 0:
            late_consts()
        if fifo_a:
            pw, psb, ptiles, pexpf = fifo_a.pop(0)
            pr, expe, g01e = phase2a(pw, psb, ptiles, pexpf)
            fifo_b.append((pw, psb, pr, expe, g01e))
        fifo_a.append((w, sb, tiles, emit_exp(w, sb, lg)))
        tick()
    while fifo_a:
        pw, psb, ptiles, pexpf = fifo_a.pop(0)
        pr, expe, g01e = phase2a(pw, psb, ptiles, pexpf)
        fifo_b.append((pw, psb, pr, expe, g01e))
    while fifo_b:
        w, sb, pr, expe, g01e = fifo_b.pop(0)
        phase2b(w, sb, pr, expe, g01e)
        if sb == _nsb(bcs[w]) - 1:
            epi_v(w)
            epi_cd.append([w, 0])
    while epi_cd:
        w = epi_cd.pop(0)[0]
        on_h(w, state.pop(("h", w)))
        del state[("win", w)], state[("U", w)]


_PROGRAM_CACHE = {}


def kernel(x, edge_index, Wl1, bl1, Wr1, br1, att1, bias1,
           Wl2, bl2, Wr2, br2, att2, bias2, Wjk, bjk):
    global LAST_RESULTS
    LAST_RESULTS = []
    trace = bool(os.environ.get("GAT_TRACE"))

    x = _f32(x)
    Wl1, Wr1 = _f32(Wl1), _f32(Wr1)
    Wl2, Wr2 = _f32(Wl2), _f32(Wr2)
    Wjk = _f32(Wjk)
    bcs, srcs, dlocs = _plan_edges(np.asarray(edge_index))
    key = tuple(bcs)

    if "A" not in _PROGRAM_CACHE:
        _PROGRAM_CACHE["A"] = _build_launch_a()
    if ("B", key) not in _PROGRAM_CACHE:
        _PROGRAM_CACHE[("B", key)] = _build_launch_b(bcs)
    if ("C", key) not in _PROGRAM_CACHE:
        _PROGRAM_CACHE[("C", key)] = _build_launch_c(bcs)

    ident = np.eye(128, dtype=np.float32)
    g01e_bf = _build_g01e(dlocs, bcs, BF16)
    g01e_f8 = _build_g01e(dlocs, bcs, F8)

    # ---------------- launch A: per-node transforms ----------------
    common_a = dict(
        Wl1p=_bf(Wl1[:, PERM1]),
        Wr1p=_bf(Wr1[:, PERM1]),
        bxr1p=_f32(np.tile((np.asarray(bl1) + np.asarray(br1))[PERM1][None, :],
                           (128, 1))),
        Wjk0=_bf(Wjk[:128]),
    )
    in_maps_a = []
    for c in range(NCORES):
        xo = np.zeros((128, NPAD), np.float32)
        xo[:, :NPC] = x[c * NPC:(c + 1) * NPC].T
        in_maps_a.append(dict(common_a, x_ownT=_bf(xo)))

    res_a = run_bass_kernel_spmd(_PROGRAM_CACHE["A"], in_maps_a,
                                 core_ids=list(range(NCORES)), trace=trace)
    LAST_RESULTS.append(res_a)

    # ---------------- host routing for layer 1 ----------------
    ao = [np.asarray(res_a.results[c]["ao"]) for c in range(NCORES)]
    xl1_all = np.concatenate([a[:NPC, :D1] for a in ao], axis=0)
    em1 = _route_edges(xl1_all, srcs, bcs, D1, with_ones=False)
    aabs1 = _aabs(np.asarray(att1))
    xl1s_f8 = (xl1_all.astype(np.float32) * aabs1[None, :]).astype(F8)
    sfx1 = _build_sfx(xl1s_f8, srcs, dlocs, bcs, nG=4)
    stream1 = _pack_stream(em1, sfx1, g01e_bf, g01e_f8, bcs, 4, D1)

    wl2i = Wl2[PERM1][:, PERM2].reshape(4, 128, D2)
    wr2i = Wr2[PERM1][:, PERM2].reshape(4, 128, D2)
    wlr2 = np.concatenate([wl2i, wr2i], axis=2)        # [4, 128, 512]
    common_b = dict(
        att1sg=_f8(_att_sg(np.asarray(att1), D1).reshape(128, 128)),
        biash1=_f32(np.tile((np.asarray(bl1) + np.asarray(bias1))[PERM1][None, :],
                            (128, 1))),
        identI=_bf(ident),
        Wlr2p=_bf(wlr2.transpose(1, 0, 2).reshape(128, 4 * 2 * D2)),
        bxr2p=_f32(np.tile((np.asarray(bl2) + np.asarray(br2))[PERM2][None, :],
                           (128, 1))),
        Wjk1p=_bf(Wjk[128:128 + D1][PERM1].reshape(4, 128, 128)
                  .transpose(1, 0, 2).reshape(128, 4 * 128)),
    )
    in_maps_b = []
    for c in range(NCORES):
        xr1 = np.zeros((NPAD, D1), np.float32)
        xr1[:NPC] = ao[c][:NPC, D1:2 * D1]
        xr1 *= aabs1[None, :]
        in_maps_b.append(dict(
            common_b,
            stream=stream1[c],
            xrl=_build_xrl(xr1.astype(F8), nG=4, with_i=True)
                .reshape(NW, 128, 4 * 2 * 128),
            jk0=np.ascontiguousarray(ao[c][:, 2 * D1:].reshape(NW, 128, 128)),
        ))

    res_b = run_bass_kernel_spmd(_PROGRAM_CACHE[("B", key)], in_maps_b,
                                 core_ids=list(range(NCORES)), trace=trace)
    LAST_RESULTS.append(res_b)

    # ---------------- host routing for layer 2 ----------------
    xl2_all = np.concatenate(
        [np.asarray(res_b.results[c]["xl2_o"])[:NPC] for c in range(NCORES)],
        axis=0)                                   # [N, 256] bf16, interleaved
    em2 = _route_edges(xl2_all, srcs, bcs, DE2, with_ones=True)
    aabs2 = _aabs(np.asarray(att2))
    xl2s_f8 = (xl2_all.astype(np.float32) * aabs2[None, :]).astype(F8)
    sfx2 = _build_sfx(xl2s_f8, srcs, dlocs, bcs, nG=2)
    stream2 = _pack_stream(em2, sfx2, None, g01e_f8, bcs, 2, DE2)

    common_c = dict(
        att2sg=_f8(_att_sg(np.asarray(att2), D2).reshape(128, 64)),
        biash2=_f32(np.tile((np.asarray(bl2) + np.asarray(bias2))[PERM2][None, :],
                            (128, 1))),
        identI=_bf(ident),
        ident8f=_f8(np.eye(8, dtype=np.float32)),
        Wjk2p=_bf(Wjk[128 + D1:][PERM2].reshape(2, 128, 128)
                  .transpose(1, 0, 2).reshape(128, 2 * 128)),
    )
    in_maps_c = []
    for c in range(NCORES):
        xr2 = np.zeros((NPAD, D2), np.float32)
        xr2[:NPC] = np.asarray(res_b.results[c]["xr2_o"])[:NPC]
        xr2 *= aabs2[None, :]
        jk01 = np.asarray(res_b.results[c]["jk01_o"]).astype(np.float32) \
            + np.asarray(bjk, np.float32)[None, :]
        in_maps_c.append(dict(
            common_c,
            stream=stream2[c],
            xrl=_build_xrl(xr2.astype(F8), nG=2, with_i=True)
                .reshape(NW, 128, 2 * 2 * 128),
            jk01=jk01.reshape(NW, 128, 128),
        ))

    res_c = run_bass_kernel_spmd(_PROGRAM_CACHE[("C", key)], in_maps_c,
                                 core_ids=list(range(NCORES)), trace=trace)
    LAST_RESULTS.append(res_c)

    out = np.concatenate(
        [np.asarray(res_c.results[c]["out_o"])[:NPC] for c in range(NCORES)],
        axis=0)
    return np.ascontiguousarray(out, dtype=np.float32)


# revision 21
# speedup vs baseline: 1.0208x; 1.0208x over previous
"""Trainium2 Bass kernel for a 2-layer GATv2 + JumpingKnowledge GNN.

Strategy (8 NeuronCores, dst-node sharding, 3 launches, zero on-device
gathers):
  - Host: add self loops, bucket edges by (core, 128-node dst window).
    Windows are padded to a per-window-position block count bc_w
    (uniform across cores so one SPMD program serves all 8).
  - Launch A (node-sharded): xl1 = x@Wl1, xr1 = x@Wr1 + biases,
    jk0 = x@Wjk0 for owned nodes.  Pure per-node GEMMs.
  - Host: route xl1 rows into edge order (halo exchange): an edge-major
    bf16 copy (messages, for the alpha-weighted aggregation) and a
    feature-major fp8 copy (for the attention-logit pipeline), plus the
    per-window fp8 scatter one-hot g01t.  Pure permutation + fp8 cast.
  - Launch B: layer-1 edge phase + h1 + layer-2 node transforms.
  - Host: route xl2 the same way.
  - Launch C: layer-2 edge phase + JumpingKnowledge output projection.

Edge phase per superblock (up to 4 blocks of 128 edges):
  s_fm[g]  = DoubleRow fp8 matmul: [xr_win_g | I]^T @ [g01t | fm_g]
             (one PE pass computes xr[dst] + xl[src] per feature group)
  lr       = Prelu(s_fm, 0.2)                          (ACT/DVE)
  lg      += att_bd[g].T @ lr[g]                       (PE; layer 2: one
             fp8 DoubleRow matmul over both groups)
  expf     = Exp(lg); expe = transpose via tiny PE matmuls
  g01e[b]  = is_equal(iota128, dloc_col[b])            (DVE one-hot gen)
  pr[b]    = em[b] * expe[b]  (head-broadcast)         (DVE 4x mode)
  U       += g01e[b].T @ pr[b]; dn += g01e[b].T @ expe (PE; layer 2:
             fp8 DoubleRow over block pairs, denominator merged into U
             via ones columns baked into em)
Window epilogue: h = elu(U/dn + bias), then next-layer node GEMMs.

fp8 (e4m3) is used only where the ~2e-2 tolerance allows: the logit-path
operands (fm, xr, one-hots) in both layers, plus the whole layer-2 edge
phase (lr, att, expe, pr).  Messages (em) and layer-1 softmax stay bf16.

All feature axes use a head-interleaved order f=(c*H+h) so the DVE
broadcast multiply has innermost stride 1 (4x DVE perf mode).  Every
weight matrix is permuted accordingly on the host; the final output is
un-permuted (Wjk rows permuted to compensate).

The segment softmax skips the max subtraction: logits for this model are
in [-6, 6], exp() is safe, softmax is shift-invariant.
"""

import os
from contextlib import ExitStack

import ml_dtypes
import numpy as np

import concourse.bacc as bacc
import concourse.mybir as mybir
import concourse.tile as tile
from concourse.bass_utils import run_bass_kernel_spmd

dt = mybir.dt
AF = mybir.ActivationFunctionType
ALU = mybir.AluOpType
BF16 = ml_dtypes.bfloat16
F8 = ml_dtypes.float8_e4m3

# ---------------- problem constants (hardcoded per contract) ----------------
N = 20000
HID = 128
HEADS = 8
C1 = 64
C2 = 32
D1 = HEADS * C1  # 512
D2 = HEADS * C2  # 256
DE2 = D2 + 8     # em2 with ones columns (merged denominator)

NCORES = 8
NPC = N // NCORES          # 2500 nodes per core
WNODES = 128               # nodes per window
NW = -(-NPC // WNODES)     # 20 windows per core
NPAD = NW * WNODES         # 2560 padded node slots per core
BLK = 128                  # edges per block
E4 = 4 * BLK               # edge slots per full superblock

LAST_RESULTS = []          # BassKernelResults of the most recent kernel() call


def _bf(x):
    return np.ascontiguousarray(np.asarray(x, np.float32).astype(BF16))


def _f8(x):
    return np.ascontiguousarray(np.asarray(x, np.float32).astype(F8))


def _f32(x):
    return np.ascontiguousarray(np.asarray(x, np.float32))


def _perm(D, H):
    """Head-interleave permutation: interleaved col j holds original col
    (j%H)*C + j//H  (i.e. j = c*H + h)."""
    j = np.arange(D)
    return (j % H) * (D // H) + j // H


PERM1 = _perm(D1, HEADS)
PERM2 = _perm(D2, HEADS)


def _att_sg(att, D):
    """Sign block-diag lhsT for the logit DoubleRow matmuls: [128, nPair,
    2, 32] fp8 (+-1 entries, zero-padded beyond 8 heads)."""
    H, C = att.shape
    nG = D // 128
    bd = np.zeros((D, 32), np.float32)
    j = np.arange(D)
    bd[j, j % H] = np.sign(att[j % H, j // H])
    return bd.reshape(nG // 2, 2, 128, 32).transpose(2, 0, 1, 3)


def _aabs(att):
    """|att| per interleaved feature column f=(c*H+h)."""
    H, C = att.shape
    j = np.arange(H * C)
    return np.abs(att[j % H, j // H]).astype(np.float32)


def _nsb(bc):
    return -(-bc // 4)


def _plan_edges(edge_index):
    """Bucket self-loop-augmented edges by (core, window).  Block counts
    bc_w are maxed over cores per window position (same SPMD program).
    Returns (bcs, srcs, dlocs) where
      bcs        = [NW] blocks per window
      srcs[c]    = [NW][bc_w*128] int64 src per slot (-1 for pads)
      dlocs[c]   = [NW][bc_w*128] int16 dst-in-window (-1 for pads)"""
    src = np.concatenate([edge_index[0].astype(np.int64),
                          np.arange(N, dtype=np.int64)])
    dst = np.concatenate([edge_index[1].astype(np.int64),
                          np.arange(N, dtype=np.int64)])
    core = dst // NPC
    dloc = dst - core * NPC
    win = dloc // WNODES
    din = dloc % WNODES

    order = np.lexsort((win, core))
    src, core, win, din = src[order], core[order], win[order], din[order]

    lists = {}
    cnts = np.zeros((NCORES, NW), np.int64)
    for c in range(NCORES):
        mc = core == c
        sc, wc, dc = src[mc], win[mc], din[mc]
        for w in range(NW):
            mw = wc == w
            lists[(c, w)] = (sc[mw], dc[mw])
            cnts[c, w] = mw.sum()
    bcs = [-(-int(cnts[:, w].max()) // BLK) for w in range(NW)]

    srcs, dlocs = [], []
    for c in range(NCORES):
        sl, dl = [], []
        for w in range(NW):
            ns = bcs[w] * BLK
            sp = np.full(ns, -1, np.int64)
            dp = np.full(ns, -1, np.int16)
            s_, d_ = lists[(c, w)]
            sp[:len(s_)] = s_
            dp[:len(d_)] = d_
            sl.append(sp)
            dl.append(dp)
        srcs.append(sl)
        dlocs.append(dl)
    return bcs, srcs, dlocs


def _route_edges(table_bf, srcs, bcs, DE, with_ones):
    """Gather table rows into edge order, per core.

    table_bf: [N, D] bf16 (feature cols already head-interleaved)
    returns (em, fm) lists per core:
      em[c]: [128, sum_w bc_w*DE] bf16, per block [128 e, DE] edge-major
             (with_ones: last 8 cols per block are 1.0; pads all-zero)
      fm[c]: [128, sum_w bc_w*nG*128... ] fp8: per sb [g01t | fm_g ...]
             is built separately in _build_sfx."""
    D = table_bf.shape[1]
    out = []
    for sl in srcs:
        pieces = []
        for w, sp in enumerate(sl):
            gat = np.zeros((len(sp), DE), BF16)
            real = sp >= 0
            gat[real, :D] = table_bf[sp[real]]
            if with_ones:
                gat[real, D:] = np.float32(1.0)
            # em[p, blk*DE + f] = gat[blk*128+p, f]
            em = gat.reshape(bcs[w], BLK, DE).transpose(1, 0, 2)
            pieces.append(np.ascontiguousarray(em).reshape(BLK, bcs[w] * DE))
        out.append(np.concatenate(pieces, axis=1))
    return out


def _build_sfx(table_f8, srcs, dlocs, bcs, nG):
    """Per-sb fp8 [g01t | fm_0 .. fm_{nG-1}] stream, per core.

    Row layout per partition p: for each (w, sb):
      [g01t[p, e], fm_0[p, e], ..., fm_{nG-1}[p, e]]  each nblk*128 wide
    g01t[n, e] = (dloc[e] == n);  fm_g[j, e] = table[src[e], g*128+j]."""
    out = []
    for sl, dl in zip(srcs, dlocs):
        pieces = []
        for w, (sp, dp) in enumerate(zip(sl, dl)):
            bc = bcs[w]
            for sb in range(_nsb(bc)):
                nblk = min(4, bc - 4 * sb)
                ne = nblk * BLK
                lo = 4 * sb * BLK
                spe, dpe = sp[lo:lo + ne], dp[lo:lo + ne]
                sfx = np.zeros((BLK, (1 + nG) * ne), F8)
                e = np.arange(ne)
                real = dpe >= 0
                sfx[dpe[real], e[real]] = np.float32(1.0)
                gat = np.zeros((ne, nG * 128), F8)
                realm = spe >= 0
                gat[realm] = table_f8[spe[realm]]
                # fm_g[j, e] = gat[e, g*128+j]
                fmt = gat.reshape(ne, nG, 128).transpose(2, 1, 0)
                sfx[:, ne:] = np.ascontiguousarray(fmt).reshape(128, nG * ne)
                pieces.append(sfx)
        out.append(np.concatenate(pieces, axis=1))
    return out


def _build_g01e(dlocs, bcs, np_dt):
    """Per-block [128 e, 128 n] one-hot (edge-major), concatenated per
    core: [128, total_blocks*128]."""
    out = []
    for dl in dlocs:
        pieces = []
        for w, dp in enumerate(dl):
            g = np.zeros((bcs[w], BLK, BLK), np_dt)
            blk = np.arange(len(dp)) // BLK
            pin = np.arange(len(dp)) % BLK
            real = dp >= 0
            g[blk[real], pin[real], dp[real]] = np.float32(1.0)
            # [p, blk*128 + n]
            pieces.append(np.ascontiguousarray(g.transpose(1, 0, 2))
                          .reshape(BLK, bcs[w] * BLK))
        out.append(np.concatenate(pieces, axis=1))
    return out


def _build_dcol(dlocs, bcs):
    """[NW, 128, 20] bf16 dst-in-window per (block, slot): dcol[w, p, b] =
    dloc of edge slot b*128+p (or -1 pad)."""
    out = []
    for dl in dlocs:
        dcol = np.full((NW, BLK, 20), -1.0, BF16)
        for w, dp in enumerate(dl):
            bc = bcs[w]
            dcol[w, :, :bc] = dp.reshape(bc, BLK).T.astype(BF16)
        out.append(dcol)
    return out


def _build_xrl(xr_f8, nG, with_i):
    """[NW, 128, nG, 2, 128] fp8 DoubleRow lhsT: k-tile0 = xr_win group g,
    k-tile1 = identity."""
    ident = np.eye(128, dtype=F8)
    out = np.zeros((NW, 128, nG, 2, 128), F8)
    for w in range(NW):
        xw = xr_f8[w * 128:(w + 1) * 128]          # [128 n, D]
        out[w, :, :, 0, :] = xw.reshape(128, nG, 128)
        out[w, :, :, 1, :] = ident[:, None, :]
    return np.ascontiguousarray(out)


# ------------------------------ launch A -----------------------------------

def _build_launch_a():
    nc = bacc.Bacc(None, target_bir_lowering=False)
    x_ownT = nc.dram_tensor("x_ownT", [128, NPAD], dt.bfloat16,
                            kind="ExternalInput")
    Wl1p = nc.dram_tensor("Wl1p", [128, D1], dt.bfloat16, kind="ExternalInput")
    Wr1p = nc.dram_tensor("Wr1p", [128, D1], dt.bfloat16, kind="ExternalInput")
    bxr1p = nc.dram_tensor("bxr1p", [128, D1], dt.float32, kind="ExternalInput")
    Wjk0 = nc.dram_tensor("Wjk0", [128, 128], dt.bfloat16, kind="ExternalInput")

    ao = nc.dram_tensor("ao", [NPAD, 2 * D1 + 128], dt.bfloat16,
                        kind="ExternalOutput")

    with tile.TileContext(nc) as tc, ExitStack() as ctx:
        const = ctx.enter_context(tc.tile_pool(name="const", bufs=1))
        sbuf = ctx.enter_context(tc.tile_pool(name="sbuf", bufs=5))
        pp = ctx.enter_context(tc.tile_pool(name="pp", bufs=6, space="PSUM"))
        pps = ctx.enter_context(tc.tile_pool(name="pps", bufs=2, space="PSUM"))

        def cl(name, hdl, shape, dtype):
            t = const.tile(shape, dtype, tag=name)
            nc.sync.dma_start(t[:], hdl[:])
            return t

        wl = cl("wl", Wl1p, [128, D1], dt.bfloat16)
        wr = cl("wr", Wr1p, [128, D1], dt.bfloat16)
        bx = cl("bx", bxr1p, [128, D1], dt.float32)
        wj = cl("wj", Wjk0, [128, 128], dt.bfloat16)
        xo = const.tile([128, NPAD], dt.bfloat16, tag="xo")
        nc.sync.dma_start(xo[:], x_ownT[:])

        outq = [nc.gpsimd, nc.gpsimd, nc.gpsimd]
        for w in range(NW):
            lhs = xo[:, w * 128:(w + 1) * 128]
            t1 = sbuf.tile([128, 2 * D1 + 128], dt.bfloat16, tag="t1")
            p1 = pp.tile([128, D1], dt.float32, tag="p1")
            nc.tensor.matmul(p1[:], lhsT=lhs, rhs=wl[:], start=True, stop=True)
            nc.vector.tensor_copy(t1[:, :D1], p1[:])
            p2 = pp.tile([128, D1], dt.float32, tag="p1")
            nc.tensor.matmul(p2[:], lhsT=lhs, rhs=wr[:], start=True, stop=True)
            nc.vector.tensor_tensor(out=t1[:, D1:2 * D1], in0=p2[:], in1=bx[:],
                                    op=ALU.add)
            p3 = pps.tile([128, 128], dt.float32, tag="p3")
            nc.tensor.matmul(p3[:], lhsT=lhs, rhs=wj[:], start=True, stop=True)
            nc.scalar.activation(t1[:, 2 * D1:], p3[:], AF.Copy)
            outq[w % 3].dma_start(ao[w * 128:(w + 1) * 128, :], t1[:])

    nc.compile()
    return nc


# ------------------------- edge-phase launches ------------------------------

def _sfx_offsets(bcs, nG):
    """Per-(w, sb) column offset of the sfx stream; total width."""
    offs = {}
    o = 0
    for w, bc in enumerate(bcs):
        for sb in range(_nsb(bc)):
            nblk = min(4, bc - 4 * sb)
            offs[(w, sb)] = o
            o += (1 + nG) * nblk * BLK
    return offs, o


def _em_offsets(bcs, DE):
    offs = {}
    o = 0
    for w, bc in enumerate(bcs):
        for sb in range(_nsb(bc)):
            nblk = min(4, bc - 4 * sb)
            offs[(w, sb)] = o
            o += nblk * DE
    return offs, o


def _build_launch_b(bcs):
    """Layer-1 edge phase: s via fp8 DoubleRow, lg/U/dn bf16, plus per-
    window h1 epilogue and layer-2 node transforms."""
    nG = 4
    sfx_off, sfx_w = _sfx_offsets(bcs, nG)
    em_off, em_w = _em_offsets(bcs, D1)
    nc = bacc.Bacc(None, target_bir_lowering=False)

    em_d = nc.dram_tensor("em", [128, em_w], dt.bfloat16, kind="ExternalInput")
    sfx_d = nc.dram_tensor("sfx", [128, sfx_w], dt.float8e4,
                           kind="ExternalInput")
    xrl_d = nc.dram_tensor("xrl", [NW, 128, nG * 2 * 128], dt.float8e4,
                           kind="ExternalInput")
    g01e_d = nc.dram_tensor("g01e", [128, sum(bcs) * BLK], dt.bfloat16,
                            kind="ExternalInput")
    jk0_d = nc.dram_tensor("jk0", [NW, 128, 128], dt.bfloat16,
                           kind="ExternalInput")
    att1sg = nc.dram_tensor("att1sg", [128, 128], dt.float8e4,
                            kind="ExternalInput")
    biash1 = nc.dram_tensor("biash1", [128, D1], dt.float32,
                            kind="ExternalInput")
    identI = nc.dram_tensor("identI", [128, 128], dt.bfloat16,
                            kind="ExternalInput")
    Wlr2p = nc.dram_tensor("Wlr2p", [128, 4 * 2 * D2], dt.bfloat16,
                           kind="ExternalInput")
    bxr2p = nc.dram_tensor("bxr2p", [128, D2], dt.float32,
                           kind="ExternalInput")
    Wjk1p = nc.dram_tensor("Wjk1p", [128, 4 * 128], dt.bfloat16,
                           kind="ExternalInput")

    xl2_o = nc.dram_tensor("xl2_o", [NPAD, D2], dt.bfloat16,
                           kind="ExternalOutput")
    xr2_o = nc.dram_tensor("xr2_o", [NPAD, D2], dt.bfloat16,
                           kind="ExternalOutput")
    jk01_o = nc.dram_tensor("jk01_o", [NPAD, 128], dt.float32,
                            kind="ExternalOutput")

    with tile.TileContext(nc) as tc, ExitStack() as ctx:
        const = ctx.enter_context(tc.tile_pool(name="const", bufs=1))
        sbuf = ctx.enter_context(tc.tile_pool(name="sbuf", bufs=3))
        empool = ctx.enter_context(tc.tile_pool(name="em", bufs=3))
        sfxpool = ctx.enter_context(tc.tile_pool(name="sfx", bufs=3))
        lrpool = ctx.enter_context(tc.tile_pool(name="lr", bufs=2))
        prpool = ctx.enter_context(tc.tile_pool(name="pr", bufs=2))
        wpool = ctx.enter_context(tc.tile_pool(name="wp", bufs=2))
        gepool = ctx.enter_context(tc.tile_pool(name="ge", bufs=4))
        ppS = ctx.enter_context(tc.tile_pool(name="ppS", bufs=2, space="PSUM"))
        ppLG = ctx.enter_context(tc.tile_pool(name="ppLG", bufs=1, space="PSUM"))
        ppE = ctx.enter_context(tc.tile_pool(name="ppE", bufs=1, space="PSUM"))
        ppU = ctx.enter_context(tc.tile_pool(name="ppU", bufs=2, space="PSUM"))
        ppDN = ctx.enter_context(tc.tile_pool(name="ppDN", bufs=2, space="PSUM"))

        def cl(name, hdl, shape, dtype, eng=None):
            t = const.tile(shape, dtype, tag=name)
            dst = t[:]
            if len(shape) > 2:
                dst = dst.rearrange("p ... -> p (...)")
            (eng or nc.sync).dma_start(dst, hdl[:])
            return t

        att1_t = cl("att1", att1sg, [128, 2, 2, 32], dt.float8e4)
        ident = cl("ident", identI, [128, 128], dt.bfloat16)
        bh1_t = const.tile([128, D1], dt.float32, tag="bh1")
        wlr2_t = const.tile([128, 4, 2 * D2], dt.bfloat16, tag="wlr2")
        bxr2_t = const.tile([128, D2], dt.float32, tag="bxr2")
        wjk1_t = const.tile([128, 4 * 128], dt.bfloat16, tag="wjk1")

        def late_consts():
            nc.gpsimd.dma_start(bh1_t[:], biash1[:])
            nc.gpsimd.dma_start(
                wlr2_t[:].rearrange("p a e -> p (a e)"), Wlr2p[:])
            nc.gpsimd.dma_start(bxr2_t[:], bxr2p[:])
            nc.gpsimd.dma_start(wjk1_t[:], Wjk1p[:])

        state = {}
        g01e_off = {}
        _o = 0
        for _w, _bc in enumerate(bcs):
            for _sb in range(_nsb(_bc)):
                g01e_off[(_w, _sb)] = _o
                _o += min(4, _bc - 4 * _sb) * BLK

        def prefetch(w, sb):
            nblk = min(4, bcs[w] - 4 * sb)
            if sb == 0:
                xrl_t = wpool.tile([128, nG, 2, 128], dt.float8e4, tag="xrl")
                nc.gpsimd.dma_start(
                    xrl_t[:].rearrange("p a b c -> p (a b c)"), xrl_d[w])
                state[("win", w)] = xrl_t
            sfx_t = sfxpool.tile([128, 1 + nG, E4], dt.float8e4, tag="sfx")
            so = sfx_off[(w, sb)]
            nc.sync.dma_start(
                sfx_t[:, :, :nblk * BLK],
                sfx_d[:, so:so + (1 + nG) * nblk * BLK]
                .rearrange("p (a e) -> p a e", a=1 + nG))
            em_t = empool.tile([128, 4, D1], dt.bfloat16, tag="em")
            eo = em_off[(w, sb)]
            nc.sync.dma_start(
                em_t[:, :nblk, :],
                em_d[:, eo:eo + nblk * D1]
                .rearrange("p (a e) -> p a e", a=nblk))
            g01e_t = gepool.tile([128, 4, BLK], dt.bfloat16, tag="g01e")
            go = g01e_off[(w, sb)]
            nc.sync.dma_start(
                g01e_t[:, :nblk, :],
                g01e_d[:, go:go + nblk * BLK]
                .rearrange("p (a e) -> p a e", a=nblk))
            return sfx_t, em_t, g01e_t

        def phase1(w, sb, tiles):
            sfx_t, em_t, g01e_t = tiles
            nblk = min(4, bcs[w] - 4 * sb)
            ne = nblk * BLK
            xrl_t = state[("win", w)]
            lg = ppLG.tile([32, E4], dt.float32, tag="lg")
            lr = lrpool.tile([128, nG, E4], dt.float8e4, tag="lr")
            for g in range(nG):
                s = ppS.tile([128, E4], dt.float32, tag="s")
                nc.tensor.matmul(
                    s[:, :ne],
                    lhsT=xrl_t[:, g, :, :],
                    rhs=sfx_t[:, 0:(2 + g):(1 + g), :ne],
                    start=True, stop=True,
                    perf_mode=mybir.MatmulPerfMode.DoubleRow)
                nc.scalar.activation(lr[:, g, :ne], s[:, :ne],
                                     AF.Prelu, alpha=0.2)
                if g == 1:
                    nc.tensor.matmul(
                        lg[:, :ne], lhsT=att1_t[:, 0, :, :],
                        rhs=lr[:, 0:2, :ne], start=True, stop=False,
                        perf_mode=mybir.MatmulPerfMode.DoubleRow)
            nc.tensor.matmul(
                lg[:, :ne], lhsT=att1_t[:, 1, :, :],
                rhs=lr[:, 2:4, :ne], start=False, stop=True,
                perf_mode=mybir.MatmulPerfMode.DoubleRow)
            return lg

        def emit_exp(w, sb, lg):
            nblk = min(4, bcs[w] - 4 * sb)
            expf = sbuf.tile([8, E4], dt.bfloat16, tag="expf")
            nc.scalar.activation(expf[:, :nblk * BLK], lg[:8, :nblk * BLK],
                                 AF.Exp)
            return expf

        def phase2a(w, sb, tiles, expf):
            _, em_t, g01e_t = tiles
            nblk = min(4, bcs[w] - 4 * sb)
            ept = ppE.tile([128, 32], dt.float32, tag="ept")
            for b in range(nblk):
                nc.tensor.matmul(ept[:, b * 8:(b + 1) * 8],
                                 lhsT=expf[:, b * BLK:(b + 1) * BLK],
                                 rhs=ident[:8, :8],
                                 start=(b == 0), stop=(b == nblk - 1))
            expe = sbuf.tile([128, 32], dt.bfloat16, tag="expe")
            nc.scalar.activation(expe[:, :nblk * 8], ept[:, :nblk * 8],
                                 AF.Copy)
            pr = prpool.tile([128, 4, D1], dt.bfloat16, tag="pr")
            for b in range(nblk):
                nc.vector.tensor_tensor(
                    out=pr[:, b, :].rearrange("p (c h) -> p c h", h=8),
                    in0=em_t[:, b, :].rearrange("p (c h) -> p c h", h=8),
                    in1=expe[:, b * 8:(b + 1) * 8].unsqueeze(1)
                        .broadcast_to([128, C1, 8]),
                    op=ALU.mult)
            return pr, expe, g01e_t

        def phase2b(w, sb, pr, expe, g01e):
            nblk = min(4, bcs[w] - 4 * sb)
            U, dn = state[("U", w)]
            nsb = _nsb(bcs[w])
            for b in range(nblk):
                first = (sb == 0 and b == 0)
                last = (sb == nsb - 1 and b == nblk - 1)
                nc.tensor.matmul(U[:], lhsT=g01e[:, b, :], rhs=pr[:, b, :],
                                 start=first, stop=last)
                nc.tensor.matmul(dn[:], lhsT=g01e[:, b, :],
                                 rhs=expe[:, b * 8:(b + 1) * 8],
                                 start=first, stop=last)

        def begin_window(w):
            U = ppU.tile([128, D1], dt.float32, tag="U")
            dn = ppDN.tile([128, 8], dt.float32, tag="dn")
            state[("U", w)] = (U, dn)

        def epi_v(w):
            U, dn = state[("U", w)]
            dns = sbuf.tile([128, 8], dt.float32, tag="dns")
            nc.vector.tensor_scalar_max(dns[:], dn[:], 1e-30)
            rd = sbuf.tile([128, 8], dt.float32, tag="rd")
            nc.vector.reciprocal(rd[:], dns[:])
            v = sbuf.tile([128, D1], dt.float32, tag="v")
            nc.vector.tensor_tensor(
                out=v[:].rearrange("p (c h) -> p c h", h=8),
                in0=U[:].rearrange("p (c h) -> p c h", h=8),
                in1=rd[:].unsqueeze(1).broadcast_to([128, C1, 8]),
                op=ALU.mult)
            vb = sbuf.tile([128, D1], dt.float32, tag="vb")
            nc.vector.tensor_tensor(out=vb[:], in0=v[:], in1=bh1_t[:],
                                    op=ALU.add)
            m = sbuf.tile([128, D1], dt.float32, tag="m")
            nc.vector.tensor_scalar_min(m[:], vb[:], 0.0)
            em_ = sbuf.tile([128, D1], dt.float32, tag="em_")
            nc.scalar.activation(em_[:], m[:], AF.Exp)
            h = sbuf.tile([128, D1], dt.bfloat16, tag="h")
            nc.vector.scalar_tensor_tensor(out=h[:], in0=em_[:], scalar=-1.0,
                                           op0=ALU.add, in1=vb[:], op1=ALU.max)
            state[("h", w)] = h

        def on_h(w, h):
            # [xl2 | xr2] = h@[Wl2p Wr2p] ; jk01 = jk0 + h@Wjk1p
            p_xlr = ppS.tile([128, 2 * D2], dt.float32, tag="s")
            p_jk = ppE.tile([128, 128], dt.float32, tag="ept")
            for g in range(4):
                tp = ppLG.tile([128, 128], dt.float32, tag="lg")
                nc.tensor.matmul(tp[:], lhsT=h[:, g * 128:(g + 1) * 128],
                                 rhs=ident[:], start=True, stop=True)
                hTs = sbuf.tile([128, 128], dt.bfloat16, tag="hT")
                nc.vector.tensor_copy(hTs[:], tp[:])
                nc.tensor.matmul(p_xlr[:], lhsT=hTs[:],
                                 rhs=wlr2_t[:, g, :],
                                 start=(g == 0), stop=(g == 3))
                nc.tensor.matmul(p_jk[:], lhsT=hTs[:],
                                 rhs=wjk1_t[:, g * 128:(g + 1) * 128],
                                 start=(g == 0), stop=(g == 3))
            o_xl2 = sbuf.tile([128, D2], dt.bfloat16, tag="oxl2")
            nc.any.tensor_copy(o_xl2[:], p_xlr[:, :D2])
            nc.gpsimd.dma_start(xl2_o[w * 128:(w + 1) * 128, :], o_xl2[:])
            o_xr2 = sbuf.tile([128, D2], dt.bfloat16, tag="oxr2")
            nc.vector.tensor_tensor(out=o_xr2[:], in0=p_xlr[:, D2:],
                                    in1=bxr2_t[:], op=ALU.add)
            nc.gpsimd.dma_start(xr2_o[w * 128:(w + 1) * 128, :], o_xr2[:])
            jk0_t = sbuf.tile([128, 128], dt.bfloat16, tag="jk0")
            nc.gpsimd.dma_start(jk0_t[:], jk0_d[w])
            o_jk = sbuf.tile([128, 128], dt.float32, tag="ojk")
            nc.vector.tensor_tensor(out=o_jk[:], in0=p_jk[:], in1=jk0_t[:],
                                    op=ALU.add)
            nc.gpsimd.dma_start(jk01_o[w * 128:(w + 1) * 128, :], o_jk[:])

        _run_pipeline(bcs, prefetch, begin_window, phase1, emit_exp,
                      phase2a, phase2b, epi_v, on_h, state, late_consts)

    nc.compile()
    return nc


def _build_launch_c(bcs):
    """Layer-2 edge phase: everything fp8 DoubleRow (s, lg, U with merged
    denominator), plus the final JK projection."""
    nG = 2
    sfx_off, sfx_w = _sfx_offsets(bcs, nG)
    em_off, em_w = _em_offsets(bcs, DE2)
    nc = bacc.Bacc(None, target_bir_lowering=False)

    em_d = nc.dram_tensor("em", [128, em_w], dt.bfloat16, kind="ExternalInput")
    sfx_d = nc.dram_tensor("sfx", [128, sfx_w], dt.float8e4,
                           kind="ExternalInput")
    xrl_d = nc.dram_tensor("xrl", [NW, 128, nG * 2 * 128], dt.float8e4,
                           kind="ExternalInput")
    g01e_d = nc.dram_tensor("g01e", [128, sum(bcs) * BLK], dt.float8e4,
                            kind="ExternalInput")
    jk01_d = nc.dram_tensor("jk01", [NW, 128, 128], dt.float32,
                            kind="ExternalInput")
    att2sg = nc.dram_tensor("att2sg", [128, 64], dt.float8e4,
                            kind="ExternalInput")
    biash2 = nc.dram_tensor("biash2", [128, D2], dt.float32,
                            kind="ExternalInput")
    identI = nc.dram_tensor("identI", [128, 128], dt.bfloat16,
                            kind="ExternalInput")
    ident8f = nc.dram_tensor("ident8f", [8, 8], dt.float8e4,
                             kind="ExternalInput")
    Wjk2p = nc.dram_tensor("Wjk2p", [128, 2 * 128], dt.bfloat16,
                           kind="ExternalInput")

    out_o = nc.dram_tensor("out_o", [NPAD, 128], dt.float32,
                           kind="ExternalOutput")

    with tile.TileContext(nc) as tc, ExitStack() as ctx:
        const = ctx.enter_context(tc.tile_pool(name="const", bufs=1))
        sbuf = ctx.enter_context(tc.tile_pool(name="sbuf", bufs=3))
        empool = ctx.enter_context(tc.tile_pool(name="em", bufs=3))
        sfxpool = ctx.enter_context(tc.tile_pool(name="sfx", bufs=3))
        lrpool = ctx.enter_context(tc.tile_pool(name="lr", bufs=2))
        prpool = ctx.enter_context(tc.tile_pool(name="pr", bufs=2))
        wpool = ctx.enter_context(tc.tile_pool(name="wp", bufs=2))
        gepool = ctx.enter_context(tc.tile_pool(name="ge", bufs=4))
        ppS = ctx.enter_context(tc.tile_pool(name="ppS", bufs=2, space="PSUM"))
        ppLG = ctx.enter_context(tc.tile_pool(name="ppLG", bufs=1, space="PSUM"))
        ppE = ctx.enter_context(tc.tile_pool(name="ppE", bufs=1, space="PSUM"))
        ppU = ctx.enter_context(tc.tile_pool(name="ppU", bufs=2, space="PSUM"))

        def cl(name, hdl, shape, dtype):
            t = const.tile(shape, dtype, tag=name)
            nc.sync.dma_start(t[:], hdl[:])
            return t

        att2_t = cl("att2", att2sg, [128, 64], dt.float8e4)
        ident = cl("ident", identI, [128, 128], dt.bfloat16)
        id8f = cl("id8f", ident8f, [8, 8], dt.float8e4)
        bh2_t = const.tile([128, D2], dt.float32, tag="bh2")
        wjk2_t = const.tile([128, 2 * 128], dt.bfloat16, tag="wjk2")

        def late_consts():
            nc.gpsimd.dma_start(bh2_t[:], biash2[:])
            nc.gpsimd.dma_start(wjk2_t[:], Wjk2p[:])

        state = {}
        g01e_off = {}
        _o = 0
        for _w, _bc in enumerate(bcs):
            for _sb in range(_nsb(_bc)):
                g01e_off[(_w, _sb)] = _o
                _o += min(4, _bc - 4 * _sb) * BLK

        def prefetch(w, sb):
            nblk = min(4, bcs[w] - 4 * sb)
            if sb == 0:
                xrl_t = wpool.tile([128, nG, 2, 128], dt.float8e4, tag="xrl")
                nc.gpsimd.dma_start(
                    xrl_t[:].rearrange("p a b c -> p (a b c)"), xrl_d[w])
                state[("win", w)] = xrl_t
            sfx_t = sfxpool.tile([128, 1 + nG, E4], dt.float8e4, tag="sfx")
            so = sfx_off[(w, sb)]
            nc.sync.dma_start(
                sfx_t[:, :, :nblk * BLK],
                sfx_d[:, so:so + (1 + nG) * nblk * BLK]
                .rearrange("p (a e) -> p a e", a=1 + nG))
            em_t = empool.tile([128, 4, DE2], dt.bfloat16, tag="em")
            eo = em_off[(w, sb)]
            nc.sync.dma_start(
                em_t[:, :nblk, :],
                em_d[:, eo:eo + nblk * DE2]
                .rearrange("p (a e) -> p a e", a=nblk))
            g01e_t = gepool.tile([128, 4, BLK], dt.float8e4, tag="g01e")
            go = g01e_off[(w, sb)]
            nc.sync.dma_start(
                g01e_t[:, :nblk, :],
                g01e_d[:, go:go + nblk * BLK]
                .rearrange("p (a e) -> p a e", a=nblk))
            return sfx_t, em_t, g01e_t

        def phase1(w, sb, tiles):
            sfx_t, em_t, g01e_t = tiles
            nblk = min(4, bcs[w] - 4 * sb)
            ne = nblk * BLK
            xrl_t = state[("win", w)]
            lr = lrpool.tile([128, nG, E4], dt.float8e4, tag="lr")
            for g in range(nG):
                s = ppS.tile([128, E4], dt.float32, tag="s")
                nc.tensor.matmul(
                    s[:, :ne],
                    lhsT=xrl_t[:, g, :, :],
                    rhs=sfx_t[:, 0:(2 + g):(1 + g), :ne],
                    start=True, stop=True,
                    perf_mode=mybir.MatmulPerfMode.DoubleRow)
                nc.scalar.activation(lr[:, g, :ne], s[:, :ne],
                                     AF.Prelu, alpha=0.2)
            lg = ppLG.tile([32, E4], dt.float32, tag="lg")
            nc.tensor.matmul(lg[:, :ne], lhsT=att2_t[:].rearrange(
                                 "p (a b) -> p a b", a=2),
                             rhs=lr[:, :, :ne], start=True, stop=True,
                             perf_mode=mybir.MatmulPerfMode.DoubleRow)
            return lg

        def emit_exp(w, sb, lg):
            nblk = min(4, bcs[w] - 4 * sb)
            expf = sbuf.tile([8, E4], dt.float8e4, tag="expf")
            nc.scalar.activation(expf[:, :nblk * BLK], lg[:8, :nblk * BLK],
                                 AF.Exp)
            return expf

        def phase2a(w, sb, tiles, expf):
            _, em_t, g01e_t = tiles
            nblk = min(4, bcs[w] - 4 * sb)
            ept = ppE.tile([128, 128], dt.float32, tag="ept")
            for b in range(nblk):
                nc.tensor.matmul(ept[:, b * 8:(b + 1) * 8],
                                 lhsT=expf[:, b * BLK:(b + 1) * BLK],
                                 rhs=id8f[:],
                                 start=(b == 0), stop=(b == nblk - 1))
            expe = sbuf.tile([128, 32], dt.float8e4, tag="expe")
            nc.scalar.activation(expe[:, :nblk * 8], ept[:, :nblk * 8],
                                 AF.Copy)
            pr = prpool.tile([128, 4, DE2], dt.float8e4, tag="pr")
            for b in range(nblk):
                nc.vector.tensor_tensor(
                    out=pr[:, b, :].rearrange("p (c h) -> p c h", h=8),
                    in0=em_t[:, b, :].rearrange("p (c h) -> p c h", h=8),
                    in1=expe[:, b * 8:(b + 1) * 8].unsqueeze(1)
                        .broadcast_to([128, C2 + 1, 8]),
                    op=ALU.mult)
            return pr, expe, g01e_t

        def phase2b(w, sb, pr, expe, g01e):
            nblk = min(4, bcs[w] - 4 * sb)
            U, _ = state[("U", w)]
            nsb = _nsb(bcs[w])
            b = 0
            while b < nblk:
                first = (sb == 0 and b == 0)
                if b + 1 < nblk:
                    last = (sb == nsb - 1 and b + 2 >= nblk)
                    nc.tensor.matmul(
                        U[:], lhsT=g01e[:, b:b + 2, :], rhs=pr[:, b:b + 2, :],
                        start=first, stop=last,
                        perf_mode=mybir.MatmulPerfMode.DoubleRow)
                    b += 2
                else:
                    last = (sb == nsb - 1)
                    nc.tensor.matmul(U[:], lhsT=g01e[:, b, :],
                                     rhs=pr[:, b, :], start=first, stop=last)
                    b += 1

        def begin_window(w):
            U = ppU.tile([128, DE2], dt.float32, tag="U")
            state[("U", w)] = (U, None)

        def epi_v(w):
            U, _ = state[("U", w)]
            dns = sbuf.tile([128, 8], dt.float32, tag="dns")
            nc.vector.tensor_scalar_max(dns[:], U[:, D2:], 1e-30)
            rd = sbuf.tile([128, 8], dt.float32, tag="rd")
            nc.vector.reciprocal(rd[:], dns[:])
            v = sbuf.tile([128, D2], dt.float32, tag="v")
            nc.vector.tensor_tensor(
                out=v[:].rearrange("p (c h) -> p c h", h=8),
                in0=U[:, :D2].rearrange("p (c h) -> p c h", h=8),
                in1=rd[:].unsqueeze(1).broadcast_to([128, C2, 8]),
                op=ALU.mult)
            vb = sbuf.tile([128, D2], dt.float32, tag="vb")
            nc.vector.tensor_tensor(out=vb[:], in0=v[:], in1=bh2_t[:],
                                    op=ALU.add)
            m = sbuf.tile([128, D2], dt.float32, tag="m")
            nc.vector.tensor_scalar_min(m[:], vb[:], 0.0)
            em_ = sbuf.tile([128, D2], dt.float32, tag="em_")
            nc.scalar.activation(em_[:], m[:], AF.Exp)
            h = sbuf.tile([128, D2], dt.bfloat16, tag="h")
            nc.vector.scalar_tensor_tensor(out=h[:], in0=em_[:], scalar=-1.0,
                                           op0=ALU.add, in1=vb[:], op1=ALU.max)
            state[("h", w)] = h

        def on_h(w, h):
            p_out = ppS.tile([128, 128], dt.float32, tag="s")
            for g in range(2):
                tp = ppLG.tile([128, 128], dt.float32, tag="lg")
                nc.tensor.matmul(tp[:], lhsT=h[:, g * 128:(g + 1) * 128],
                                 rhs=ident[:], start=True, stop=True)
                hTs = sbuf.tile([128, 128], dt.bfloat16, tag="hT")
                nc.vector.tensor_copy(hTs[:], tp[:])
                nc.tensor.matmul(p_out[:], lhsT=hTs[:],
                                 rhs=wjk2_t[:, g * 128:(g + 1) * 128],
                                 start=(g == 0), stop=(g == 1))
            jk_t = sbuf.tile([128, 128], dt.float32, tag="jkt")
            nc.gpsimd.dma_start(jk_t[:], jk01_d[w])
            o_t = sbuf.tile([128, 128], dt.float32, tag="ot")
            nc.vector.tensor_tensor(out=o_t[:], in0=p_out[:], in1=jk_t[:],
                                    op=ALU.add)
            nc.gpsimd.dma_start(out_o[w * 128:(w + 1) * 128, :], o_t[:])

        _run_pipeline(bcs, prefetch, begin_window, phase1, emit_exp,
                      phase2a, phase2b, epi_v, on_h, state, late_consts)

    nc.compile()
    return nc


def _run_pipeline(bcs, prefetch, begin_window, phase1, emit_exp, phase2a,
                  phase2b, epi_v, on_h, state, late_consts):
    """Software pipeline: phase2a lags phase1 by 1 superblock, phase2b by
    3, window epilogue 2 iterations after the last phase2b."""
    items = [(w, sb) for w in range(NW) for sb in range(_nsb(bcs[w]))]
    tiles_q = []
    fifo_a, fifo_b = [], []
    epi_cd = []

    tiles_q.append(prefetch(*items[0]))

    def tick():
        if len(fifo_b) > 2:
            w, sb, pr, expe, g01e = fifo_b.pop(0)
            phase2b(w, sb, pr, expe, g01e)
            if sb == _nsb(bcs[w]) - 1:
                epi_v(w)
                epi_cd.append([w, 2])
        for e in epi_cd:
            e[1] -= 1
        while epi_cd and epi_cd[0][1] <= 0:
            w = epi_cd.pop(0)[0]
            on_h(w, state.pop(("h", w)))
            del state[("win", w)], state[("U", w)]

    for i, (w, sb) in enumerate(items):
        if sb == 0:
            begin_window(w)
        if i + 1 < len(items):
            tiles_q.append(prefetch(*items[i + 1]))
        tiles = tiles_q.pop(0)
        lg = phase1(w, sb, tiles)
        if i == 0:
            late_consts()
        if fifo_a:
            pw, psb, ptiles, pexpf = fifo_a.pop(0)
            pr, expe, g01e = phase2a(pw, psb, ptiles, pexpf)
            fifo_b.append((pw, psb, pr, expe, g01e))
        fifo_a.append((w, sb, tiles, emit_exp(w, sb, lg)))
        tick()
    while fifo_a:
        pw, psb, ptiles, pexpf = fifo_a.pop(0)
        pr, expe, g01e = phase2a(pw, psb, ptiles, pexpf)
        fifo_b.append((pw, psb, pr, expe, g01e))
    while fifo_b:
        w, sb, pr, expe, g01e = fifo_b.pop(0)
        phase2b(w, sb, pr, expe, g01e)
        if sb == _nsb(bcs[w]) - 1:
            epi_v(w)
            epi_cd.append([w, 0])
    while epi_cd:
        w = epi_cd.pop(0)[0]
        on_h(w, state.pop(("h", w)))
        del state[("win", w)], state[("U", w)]


_PROGRAM_CACHE = {}


def kernel(x, edge_index, Wl1, bl1, Wr1, br1, att1, bias1,
           Wl2, bl2, Wr2, br2, att2, bias2, Wjk, bjk):
    global LAST_RESULTS
    LAST_RESULTS = []
    trace = bool(os.environ.get("GAT_TRACE"))

    x = _f32(x)
    Wl1, Wr1 = _f32(Wl1), _f32(Wr1)
    Wl2, Wr2 = _f32(Wl2), _f32(Wr2)
    Wjk = _f32(Wjk)
    bcs, srcs, dlocs = _plan_edges(np.asarray(edge_index))
    key = tuple(bcs)

    if "A" not in _PROGRAM_CACHE:
        _PROGRAM_CACHE["A"] = _build_launch_a()
    if ("B", key) not in _PROGRAM_CACHE:
        _PROGRAM_CACHE[("B", key)] = _build_launch_b(bcs)
    if ("C", key) not in _PROGRAM_CACHE:
        _PROGRAM_CACHE[("C", key)] = _build_launch_c(bcs)

    ident = np.eye(128, dtype=np.float32)
    g01e_bf = _build_g01e(dlocs, bcs, BF16)
    g01e_f8 = _build_g01e(dlocs, bcs, F8)

    # ---------------- launch A: per-node transforms ----------------
    common_a = dict(
        Wl1p=_bf(Wl1[:, PERM1]),
        Wr1p=_bf(Wr1[:, PERM1]),
        bxr1p=_f32(np.tile((np.asarray(bl1) + np.asarray(br1))[PERM1][None, :],
                           (128, 1))),
        Wjk0=_bf(Wjk[:128]),
    )
    in_maps_a = []
    for c in range(NCORES):
        xo = np.zeros((128, NPAD), np.float32)
        xo[:, :NPC] = x[c * NPC:(c + 1) * NPC].T
        in_maps_a.append(dict(common_a, x_ownT=_bf(xo)))

    res_a = run_bass_kernel_spmd(_PROGRAM_CACHE["A"], in_maps_a,
                                 core_ids=list(range(NCORES)), trace=trace)
    LAST_RESULTS.append(res_a)

    # ---------------- host routing for layer 1 ----------------
    ao = [np.asarray(res_a.results[c]["ao"]) for c in range(NCORES)]
    xl1_all = np.concatenate([a[:NPC, :D1] for a in ao], axis=0)
    em1 = _route_edges(xl1_all, srcs, bcs, D1, with_ones=False)
    aabs1 = _aabs(np.asarray(att1))
    xl1s_f8 = (xl1_all.astype(np.float32) * aabs1[None, :]).astype(F8)
    sfx1 = _build_sfx(xl1s_f8, srcs, dlocs, bcs, nG=4)
    stream1 = _pack_stream(em1, sfx1, g01e_bf, g01e_f8, bcs, 4, D1)

    wl2i = Wl2[PERM1][:, PERM2].reshape(4, 128, D2)
    wr2i = Wr2[PERM1][:, PERM2].reshape(4, 128, D2)
    wlr2 = np.concatenate([wl2i, wr2i], axis=2)        # [4, 128, 512]
    common_b = dict(
        att1sg=_f8(_att_sg(np.asarray(att1), D1).reshape(128, 128)),
        biash1=_f32(np.tile((np.asarray(bl1) + np.asarray(bias1))[PERM1][None, :],
                            (128, 1))),
        identI=_bf(ident),
        Wlr2p=_bf(wlr2.transpose(1, 0, 2).reshape(128, 4 * 2 * D2)),
        bxr2p=_f32(np.tile((np.asarray(bl2) + np.asarray(br2))[PERM2][None, :],
                           (128, 1))),
        Wjk1p=_bf(Wjk[128:128 + D1][PERM1].reshape(4, 128, 128)
                  .transpose(1, 0, 2).reshape(128, 4 * 128)),
    )
    in_maps_b = []
    for c in range(NCORES):
        xr1 = np.zeros((NPAD, D1), np.float32)
        xr1[:NPC] = ao[c][:NPC, D1:2 * D1]
        xr1 *= aabs1[None, :]
        in_maps_b.append(dict(
            common_b,
            stream=stream1[c],
            xrl=_build_xrl(xr1.astype(F8), nG=4, with_i=True)
                .reshape(NW, 128, 4 * 2 * 128),
            jk0=np.ascontiguousarray(ao[c][:, 2 * D1:].reshape(NW, 128, 128)),
        ))

    res_b = run_bass_kernel_spmd(_PROGRAM_CACHE[("B", key)], in_maps_b,
                                 core_ids=list(range(NCORES)), trace=trace)
    LAST_RESULTS.append(res_b)

    # ---------------- host routing for layer 2 ----------------
    xl2_all = np.concatenate(
        [np.asarray(res_b.results[c]["xl2_o"])[:NPC] for c in range(NCORES)],
        axis=0)                                   # [N, 256] bf16, interleaved
    em2 = _route_edges(xl2_all, srcs, bcs, DE2, with_ones=True)
    aabs2 = _aabs(np.asarray(att2))
    xl2s_f8 = (xl2_all.astype(np.float32) * aabs2[None, :]).astype(F8)
    sfx2 = _build_sfx(xl2s_f8, srcs, dlocs, bcs, nG=2)
    stream2 = _pack_stream(em2, sfx2, None, g01e_f8, bcs, 2, DE2)

    common_c = dict(
        att2sg=_f8(_att_sg(np.asarray(att2), D2).reshape(128, 64)),
        biash2=_f32(np.tile((np.asarray(bl2) + np.asarray(bias2))[PERM2][None, :],
                            (128, 1))),
        identI=_bf(ident),
        ident8f=_f8(np.eye(8, dtype=np.float32)),
        Wjk2p=_bf(Wjk[128 + D1:][PERM2].reshape(2, 128, 128)
                  .transpose(1, 0, 2).reshape(128, 2 * 128)),
    )
    in_maps_c = []
    for c in range(NCORES):
        xr2 = np.zeros((NPAD, D2), np.float32)
        xr2[:NPC] = np.asarray(res_b.results[c]["xr2_o"])[:NPC]
        xr2 *= aabs2[None, :]
        jk01 = np.asarray(res_b.results[c]["jk01_o"]).astype(np.float32) \
            + np.asarray(bjk, np.float32)[None, :]
        in_maps_c.append(dict(
            common_c,
            stream=stream2[c],
            xrl=_build_xrl(xr2.astype(F8), nG=2, with_i=True)
                .reshape(NW, 128, 2 * 2 * 128),
            jk01=jk01.reshape(NW, 128, 128),
        ))

    res_c = run_bass_kernel_spmd(_PROGRAM_CACHE[("C", key)], in_maps_c,
                                 core_ids=list(range(NCORES)), trace=trace)
    LAST_RESULTS.append(res_c)

    out = np.concatenate(
        [np.asarray(res_c.results[c]["out_o"])[:NPC] for c in range(NCORES)],
        axis=0)
    return np.ascontiguousarray(out, dtype=np.float32)
